# revision 53
# baseline (speedup 1.0000x reference)
"""Deformable Conv2d (DCNv2-style) Trainium2 Bass kernel.

Shards over 8 NeuronCores: core = b * 2 + ph  (b = batch 0..3, ph = pixel half).
Each core computes output pixels [ph*2048, (ph+1)*2048) of batch b.

Device pipeline per core (x-gather scheme; samples raw x rows, so there is
no U staging and the 52us of gather DMA is the only large HBM stream):
  1. offset/mask 3x3 conv as 9 shifted-tap matmuls (PE, f32, PSUM
     accumulate) over a host-padded input; junk PE transposes beforehand
     keep the cost model's PE p-state ramp at full clock for the conv
  2. B-order ([P, pixel P*16+g]) conv rows assembled via [18, 64]
     transposes per chunk-pair; B pipeline computes gather row indices
     (floor via RNE int cast, -0.5 baked into host tables); idx wrap
     ([16, n/16]) via PE transposes, then a 3-part DRAM bounce replicates
     it to 128 partitions so the first gathers start early.  A pipeline
     (bilinear coefficients cT, pixel-major) overlaps the first gathers.
  3. per tap: dma_gather of 4096 row-pair descriptors (512B) from the
     host-staged x^T halo (xg [UR*128] f16); the next tap's Pool-side
     descriptor prep is issued ahead of this tap's Pool scale ops so the
     DMA engines stay back-to-back; the last tap is split by y-pair to
     shorten the drain
  4. per group: scale the 4 bilinear corners (12 ACT / 8 Pool / 44 DVE
     per tap, per-partition scalars from cT); transpose-accumulate the
     scaled [pix, c] tiles into rotating PSUM quarters sT[c, pix] via
     matmul-with-identity (fuses corner-sum + transpose)
  5. per quarter: sT -> SBUF f16 (DVE/ACT), out[o, pix] += W_k^T @ sT
     accumulated in PSUM over the 9 taps
  6. store channel-major [128, 2048] f32 per quarter; host adds bias and
     reassembles [O, 32, 64] per core.
"""
import sys

sys.path.insert(0, "/opt/trn_rl_repo")

import numpy as np

import concourse.mybir as mybir
from concourse.ap import AP
from concourse.bacc import Bacc
from concourse.tile import TileContext
from concourse import bass_utils

F32 = mybir.dt.float32
F16 = mybir.dt.float16
I32 = mybir.dt.int32
I16 = mybir.dt.int16
Alu = mybir.AluOpType
Act = mybir.ActivationFunctionType

B, C, H, W = 4, 128, 64, 64
O, K, KK = 128, 3, 9
HWp = H * W
HALF = HWp // 2              # 2048 pixels per core
HROWS = 32
XR = 38                      # local halo rows: global [h0-3, h0+35); |dy|<2 safe
XPIX = XR * W                # 2560
UR = XPIX + 2                # xg rows (zero rows 0 and UR-1)
G16 = HALF // 128            # 16 pixel groups
XPD = 34 * 66                # host-padded conv input cols


def _colsA(tile, row0, nrow=9):
    """A-pipeline view: [128, 16g x nrow] cols of cP (stride 27)."""
    t = tile[:, :]
    return AP(tensor=t.tensor, offset=t.offset + row0,
              ap=[t.ap[0], [27, G16], [1, nrow]])


def _colsB(tile, row0, nrow=9):
    t = tile[:, :]
    return AP(tensor=t.tensor, offset=t.offset + row0,
              ap=[t.ap[0], [18, G16], [1, nrow]])


def _build(nc, debug=False):
    xp = nc.dram_tensor("xp", [C, XPD], F16, kind="ExternalInput")
    xg = nc.dram_tensor("xg", [UR * 128], F16, kind="ExternalInput")
    wc = nc.dram_tensor("wc", [C, 9 * 27], F16, kind="ExternalInput")
    bvec = nc.dram_tensor("bvec", [27, 1], F32, kind="ExternalInput")
    wkt = nc.dram_tensor("wkt", [C, KK * O], F16, kind="ExternalInput")
    gyA = nc.dram_tensor("gyA", [128, 144], F32, kind="ExternalInput")
    gxA = nc.dram_tensor("gxA", [128, 144], F32, kind="ExternalInput")
    gyB = nc.dram_tensor("gyB", [128, 144], F32, kind="ExternalInput")
    gxB = nc.dram_tensor("gxB", [128, 144], F32, kind="ExternalInput")
    shiftv = nc.dram_tensor("shiftv", [128, 2], F32, kind="ExternalInput")
    ident = nc.dram_tensor("ident", [128, 128], F16, kind="ExternalInput")
    identf = nc.dram_tensor("identf", [128, 128], F32, kind="ExternalInput")
    out = nc.dram_tensor("out", [O, HALF], F32, kind="ExternalOutput")

    idxstage = nc.dram_tensor("idxstage", [16 * 18 * 128], I16, kind="Internal")

    if debug:
        dbg_conv = nc.dram_tensor("dbg_conv", [27, HALF], F32, kind="ExternalOutput")
        dbg_cT = nc.dram_tensor("dbg_cT", [128, G16 * 36], F32, kind="ExternalOutput")
        dbg_wrap = nc.dram_tensor("dbg_wrap", [16, 18 * 128], I16, kind="ExternalOutput")

    with TileContext(nc) as tc:
        with (
            tc.tile_pool(name="big", bufs=1) as big,
            tc.tile_pool(name="small", bufs=1) as small,
        ):
            # idents first on the sync queue: the PE p-state filler is gated
            # only on idf_sb, so it must arrive as early as possible
            idf_sb = small.tile([128, 128], F32, tag="identf")
            nc.sync.dma_start(idf_sb[:, :], identf[:, :])
            id_sb = small.tile([128, 128], F16, tag="ident")
            nc.sync.dma_start(id_sb[:, :], ident[:, :])
            xpad = big.tile([C, XPD], F16, tag="xpad")
            nc.gpsimd.dma_start(xpad[:, :], xp[:, :])
            wc_sb = small.tile([C, 9 * 27], F16, tag="wc")
            nc.scalar.dma_start(wc_sb[:, :], wc[:, :])
            bvec_sb = small.tile([27, 1], F32, tag="bvec")
            nc.scalar.dma_start(bvec_sb[:, :], bvec[:, :])
            gyB_sb = small.tile([128, 144], F32, tag="gyB")
            nc.scalar.dma_start(gyB_sb[:, :], gyB[:, :])
            gxB_sb = small.tile([128, 144], F32, tag="gxB")
            nc.scalar.dma_start(gxB_sb[:, :], gxB[:, :])
            shift_sb = small.tile([128, 2], F32, tag="shiftv")
            nc.scalar.dma_start(shift_sb[:, :], shiftv[:, :])
            wk_sb = big.tile([C, KK * O], F16, tag="wk")
            nc.sync.dma_start(wk_sb[:, :], wkt[:, :])
            gyA_sb = small.tile([128, 144], F32, tag="gyA")
            nc.sync.dma_start(gyA_sb[:, :], gyA[:, :])
            gxA_sb = small.tile([128, 144], F32, tag="gxA")
            nc.sync.dma_start(gxA_sb[:, :], gxA[:, :])
            # dummy sigmoid (memset input, no DMA dep) so the first ACT
            # table load picks a set containing sigmoid+identity+copy
            sgdum = small.tile([27, 2], F32, tag="sgdum")
            nc.vector.memset(sgdum[:, 0:1], 0.0)
            nc.scalar.activation(sgdum[:, 1:2], sgdum[:, 0:1], Act.Sigmoid)

            # ---------- offset/mask conv (A order: col = pixel) ----------
            convR = big.tile([27, HALF], F32, tag="convR")
            with (
                tc.tile_pool(name="pfconv", bufs=2, space="PSUM") as psf,
                tc.tile_pool(name="pfb", bufs=1, space="PSUM") as psb,
                tc.tile_pool(name="pftr", bufs=4, space="PSUM") as psc,
            ):
                # PE p-state filler: junk transposes gated only on the early
                # ident load, keeping the PE busy-run start well before the
                # conv release so the conv matmuls dispatch at full clock.
                for _ in range(17):
                    pjunk = psc.tile([128, 128], F32, tag="tx")
                    nc.tensor.transpose(pjunk[0:16, :], idf_sb[:, 0:16],
                                        idf_sb[:, :])
                cRv = convR[:, :]
                # B-order rows: ptB_h[h][p64, g*18+row] = conv value (row)
                # at pixel (64h+p64)*16+g; filled per conv chunk-pair by
                # [18, 64] transposes (transpose outs must sit at PSUM
                # partition 0), then assembled into SBUF cPB by two copies.
                ptB0 = psb.tile([64, G16 * 18], F32, tag="ptB0")
                ptB1 = psb.tile([64, G16 * 18], F32, tag="ptB1")
                cPB = big.tile([128, G16 * 18], F32, tag="cPB")
                cP = big.tile([128, G16 * 27], F32, tag="cP")

                def emit_btrans(h2):
                    pt_h = ptB0 if h2 == 0 else ptB1
                    for g in range(G16):
                        nc.tensor.transpose(
                            pt_h[:, g * 18:(g + 1) * 18],
                            AP(tensor=cRv.tensor,
                               offset=cRv.offset + 1024 * h2 + g,
                               ap=[[cRv.ap[0][0], 18], [16, 64]]),
                            idf_sb[0:18, 0:18])
                    nc.vector.tensor_copy(
                        cPB[64 * h2:64 * h2 + 64, :], pt_h[:, :])

                def emit_atrans(ch):
                    for g in range(4 * ch, 4 * ch + 4):
                        pt = psc.tile([128, 128], F32, tag="tx")
                        nc.tensor.transpose(
                            pt[:, 0:27], convR[:, g * 128:(g + 1) * 128],
                            idf_sb[0:27, 0:27])
                        nc.scalar.activation(cP[:, g * 27:(g + 1) * 27],
                                             pt[:, 0:27], Act.Copy)

                for ch in range(4):
                    pc = psf.tile([27, 512], F32, tag="pf")
                    for th in range(3):
                        for tw in range(3):
                            tap = th * 3 + tw
                            rhs = AP(
                                tensor=xpad.tensor,
                                offset=xpad[:, :].offset + (ch * 8 + th) * 66 + tw,
                                ap=[xpad[:, :].ap[0], [66, 8], [1, W]],
                            )
                            nc.tensor.matmul(
                                pc[:, :], wc_sb[:, tap * 27:(tap + 1) * 27], rhs,
                                start=(tap == 0), stop=(tap == 8),
                            )
                    nc.scalar.activation(
                        convR[0:27, ch * 512:(ch + 1) * 512], pc[:, :], Act.Identity,
                        bias=bvec_sb[:, 0:1], scale=1.0,
                    )
                    if ch == 2:
                        emit_btrans(0)
                emit_btrans(1)

                # ---------- B pipeline: gather indices ----------
                bw = big.tile([128, 8 * 144], F32, tag="bw")
                idxPM = big.tile([128, 288], F32, tag="idxPM")
                itmpB = small.tile([128, 144], I32, tag="itmpB")
                itmpB2 = small.tile([128, 144], I32, tag="itmpB2")

                def Sb(q):
                    return bw[:, q * 144:(q + 1) * 144]

                BPY, BPX, BY0, BX0, BT, BIX, BCY, BT2 = range(8)
                # gyB/gxB tables have -0.5 pre-baked (RNE int cast = floor)
                nc.vector.tensor_tensor(Sb(BPY), _colsB(cPB, 0), gyB_sb[:, :], Alu.add)
                nc.vector.tensor_tensor(Sb(BPX), _colsB(cPB, 9), gxB_sb[:, :], Alu.add)
                nc.vector.tensor_copy(itmpB[:, :], Sb(BPY))
                nc.vector.tensor_copy(Sb(BY0), itmpB[:, :])
                nc.vector.tensor_copy(itmpB2[:, :], Sb(BPX))
                nc.vector.tensor_copy(Sb(BX0), itmpB2[:, :])
                nc.vector.tensor_scalar(Sb(BIX), Sb(BX0), -1.0, 63.0, Alu.max, Alu.min)

                def idx_view(pair):
                    t = idxPM[:, :]
                    return AP(tensor=t.tensor, offset=t.offset + pair * 144,
                              ap=[t.ap[0], [1, G16], [16, 9]])

                # per k-third (k0-2 | k3-5 | k6-8) so wraps/bounce start early
                def ksl(base, kg):
                    t = bw[:, :]
                    return AP(tensor=t.tensor,
                              offset=t.offset + base * 144 + kg * 3,
                              ap=[t.ap[0], [9, G16], [1, 3]])

                def idx_view_k(pair, kg):
                    t = idxPM[:, :]
                    return AP(tensor=t.tensor,
                              offset=t.offset + pair * 144 + kg * 48,
                              ap=[t.ap[0], [1, G16], [16, 3]])
                # idxPM col = pair*144 + k*16 + g ; k = 3kg+j -> offset 48kg ✓

                for kg in range(3):
                    # idx0 = clamp(y0)*64 + shift + ix
                    nc.vector.tensor_scalar(ksl(BCY, kg), ksl(BY0, kg),
                                            0.0, 63.0, Alu.max, Alu.min)
                    nc.vector.tensor_scalar(ksl(BT2, kg), ksl(BCY, kg), 64.0,
                                            shift_sb[:, 0:1], Alu.mult, Alu.add)
                    nc.vector.tensor_tensor(ksl(BT2, kg), ksl(BT2, kg),
                                            ksl(BIX, kg), Alu.add)
                    nc.vector.tensor_scalar(idx_view_k(0, kg), ksl(BT2, kg),
                                            0.0, float(UR - 2), Alu.max, Alu.min)
                    # idx1 = clamp(y0,-1,62)*64 + (shift+64) + ix
                    nc.vector.tensor_scalar(ksl(BCY, kg), ksl(BY0, kg),
                                            -1.0, 62.0, Alu.max, Alu.min)
                    nc.vector.tensor_scalar(ksl(BT2, kg), ksl(BCY, kg), 64.0,
                                            shift_sb[:, 1:2], Alu.mult, Alu.add)
                    nc.vector.tensor_tensor(ksl(BT2, kg), ksl(BT2, kg),
                                            ksl(BIX, kg), Alu.add)
                    nc.vector.tensor_scalar(idx_view_k(1, kg), ksl(BT2, kg),
                                            0.0, float(UR - 2), Alu.max, Alu.min)

                # idx transposes -> wrap rows [16, 128] each, cast to i16
                wrapS = big.tile([16, 18 * 128], I16, tag="wrapS")
                for k in range(KK):
                    for pair in range(2):
                        pw = psc.tile([128, 128], F32, tag="tx")
                        nc.tensor.transpose(
                            pw[0:16, :],
                            idxPM[:, pair * 144 + k * 16: pair * 144 + (k + 1) * 16],
                            idf_sb[:, :])
                        r = k * 2 + pair
                        if pair == 0:
                            nc.vector.tensor_copy(
                                wrapS[:, r * 128:(r + 1) * 128], pw[0:16, :])
                        else:
                            nc.scalar.activation(
                                wrapS[:, r * 128:(r + 1) * 128], pw[0:16, :], Act.Copy)
                if debug:
                    nc.sync.dma_start(dbg_wrap[:, :], wrapS[:, :])
                # bounce to DRAM and back replicated x8, in 3 k-thirds
                # so the first gathers start as early as possible
                idxW = big.tile([128, 18 * 128], I16, tag="idxW")
                for kg in range(3):
                    c0, c1 = kg * 768, (kg + 1) * 768
                    nc.sync.dma_start(
                        AP(tensor=idxstage, offset=c0,
                           ap=[[2304, 16], [1, 768]]),
                        wrapS[:, c0:c1])
                    nc.scalar.dma_start(
                        idxW[:, c0:c1],
                        AP(tensor=idxstage, offset=c0,
                           ap=[[0, 8], [2304, 16], [1, 768]]))

                # ---------- A pipeline: coefficients (pixel-major) ----------
                for ch in range(4):
                    emit_atrans(ch)

                NSL = 20
                cw = big.tile([128, NSL * 144], F32, tag="cw")
                itmp = small.tile([128, 144], I32, tag="itmp")
                cT = big.tile([128, G16 * 36], F32, tag="cT")

                def S(q):
                    return cw[:, q * 144:(q + 1) * 144]

                PY, PX, M, Y0, X0, FY, FX, Y1, X1 = range(9)
                CY0, CY1, VY0, VY1, VX0, VX1, IXC, T1, T2, T3 = range(9, 19)
                nc.vector.tensor_tensor(S(PY), _colsA(cP, 0), gyA_sb[:, :], Alu.add)
                nc.vector.tensor_tensor(S(PX), _colsA(cP, 9), gxA_sb[:, :], Alu.add)
                nc.scalar.activation(S(M), _colsA(cP, 18), Act.Sigmoid)
                # floors
                nc.vector.tensor_scalar(S(T1), S(PY), -0.5, None, Alu.add)
                nc.vector.tensor_copy(itmp[:, :], S(T1))
                nc.vector.tensor_copy(S(Y0), itmp[:, :])
                nc.vector.tensor_scalar(S(T1), S(PX), -0.5, None, Alu.add)
                nc.vector.tensor_copy(itmp[:, :], S(T1))
                nc.vector.tensor_copy(S(X0), itmp[:, :])
                nc.vector.tensor_tensor(S(FY), S(PY), S(Y0), Alu.subtract)
                nc.vector.tensor_tensor(S(FX), S(PX), S(X0), Alu.subtract)
                nc.vector.tensor_scalar(S(Y1), S(Y0), 1.0, None, Alu.add)
                nc.vector.tensor_scalar(S(X1), S(X0), 1.0, None, Alu.add)
                # validity
                nc.vector.tensor_scalar(S(CY0), S(Y0), 0.0, 63.0, Alu.max, Alu.min)
                nc.vector.tensor_tensor(S(VY0), S(CY0), S(Y0), Alu.is_equal)
                nc.vector.tensor_scalar(S(CY1), S(Y1), 0.0, 63.0, Alu.max, Alu.min)
                nc.vector.tensor_tensor(S(VY1), S(CY1), S(Y1), Alu.is_equal)
                nc.vector.tensor_scalar(S(T1), S(X0), 0.0, 63.0, Alu.max, Alu.min)
                nc.vector.tensor_tensor(S(VX0), S(T1), S(X0), Alu.is_equal)
                nc.vector.tensor_scalar(S(T1), S(X1), 0.0, 63.0, Alu.max, Alu.min)
                nc.vector.tensor_tensor(S(VX1), S(T1), S(X1), Alu.is_equal)
                # weights: wy0=(1-fy)*m*vy0 ; wy1=fy*m*vy1 ; ax0=(1-fx)*vx0 ; ax1=fx*vx1
                nc.vector.tensor_scalar(S(T1), S(FY), -1.0, 1.0, Alu.mult, Alu.add)
                nc.vector.tensor_tensor(S(T1), S(T1), S(M), Alu.mult)
                nc.vector.tensor_tensor(S(T1), S(T1), S(VY0), Alu.mult)     # wy0
                nc.vector.tensor_tensor(S(T2), S(FY), S(M), Alu.mult)
                nc.vector.tensor_tensor(S(T2), S(T2), S(VY1), Alu.mult)     # wy1
                nc.vector.tensor_scalar(S(T3), S(FX), -1.0, 1.0, Alu.mult, Alu.add)
                nc.vector.tensor_tensor(S(T3), S(T3), S(VX0), Alu.mult)     # ax0
                nc.vector.tensor_tensor(S(FX), S(FX), S(VX1), Alu.mult)     # ax1

                def cT_view(corner):
                    t = cT[:, :]
                    return AP(tensor=t.tensor, offset=t.offset + corner * 9,
                              ap=[t.ap[0], [36, G16], [1, 9]])

                nc.vector.tensor_tensor(cT_view(0), S(T1), S(T3), Alu.mult)  # c00
                nc.vector.tensor_tensor(cT_view(1), S(T1), S(FX), Alu.mult)  # c01
                nc.vector.tensor_tensor(cT_view(2), S(T2), S(T3), Alu.mult)  # c10
                nc.vector.tensor_tensor(cT_view(3), S(T2), S(FX), Alu.mult)  # c11
                if debug:
                    nc.sync.dma_start(dbg_cT[:, :], cT[:, :])

            # ---------- gathers + combine ----------
            src_ap = AP(tensor=xg, offset=0, ap=[[128, UR - 1], [1, 256]])
            with (
                tc.tile_pool(name="gat", bufs=4) as gat,
                tc.tile_pool(name="tmul", bufs=10) as tmul,
                tc.tile_pool(name="pst", bufs=4, space="PSUM") as pst,
                tc.tile_pool(name="pso", bufs=1, space="PSUM") as pso,
                tc.tile_pool(name="stsb", bufs=3) as stsb,
                tc.tile_pool(name="osb", bufs=1) as osbp,
            ):
                po = pso.tile([128, HALF], F32, tag="po")
                osb = osbp.tile([128, HALF], F32, tag="osb")
                gts = []

                def emit_gather(k):
                    gt = gat.tile([128, 2 * G16, 256], F16, tag="gt")
                    nc.gpsimd.dma_gather(
                        gt[:, :, :], src_ap,
                        idxW[:, k * 256:(k + 1) * 256],
                        2 * HALF, 2 * HALF, 256, elem_step=128,
                        single_packet=False)
                    gts.append(gt)

                def emit_gather8(pair):
                    # last tap split by y-corner pair: its combine can start
                    # after the first half lands, shortening the tail
                    gt = gat.tile([128, G16, 256], F16, tag="gt8")
                    c0 = (KK - 1) * 256 + pair * 128
                    nc.gpsimd.dma_gather(
                        gt[:, :, :], src_ap,
                        idxW[:, c0:c0 + 128],
                        HALF, HALF, 256, elem_step=128,
                        single_packet=False)
                    gts.append(gt)

                emit_gather(0)
                for k in range(KK):
                    # issue the next gather's Pool prep before this tap's
                    # Pool scale ops so the DMA cadence never head-of-line
                    # blocks on data-dependent work
                    if k + 1 < KK - 1:
                        emit_gather(k + 1)
                    elif k + 1 == KK - 1:
                        emit_gather8(0)
                        emit_gather8(1)
                    gt = gts[k]
                    st16 = stsb.tile([128, HALF], F16, tag="st16")
                    tbs = []
                    for _c in range(4):
                        tb = tmul.tile([128, HALF], F16, tag="tb")
                        tbs.append(tb)
                    # per group: scale the 4 corners (ACT first, then DVE),
                    # transpose-accumulate into the quarter's sT[c, pix] via
                    # matmul-with-identity (consecutive accumulation group);
                    # per quarter: copy sT -> f16 and run the W matmul.
                    # sT quarters rotate through 4 PSUM banks so tap k+1's
                    # transposes only wait on this tap's same-quarter copy.
                    sTq = None
                    sTqs = [None] * 4
                    for g in range(G16):
                        if g % 4 == 0:
                            sTq = pst.tile([128, 512], F32, tag="sTq")
                        for corner in (0, 1, 2, 3):
                            sc = cT[:, g * 36 + corner * 9 + k:
                                    g * 36 + corner * 9 + k + 1]
                            dst = tbs[corner][:, g * 128:(g + 1) * 128]
                            pair, half = corner // 2, corner % 2
                            if k == KK - 1:
                                srcg = gts[KK - 1 + pair][:, g,
                                           half * 128:(half + 1) * 128]
                            else:
                                srcg = gt[:, pair * G16 + g,
                                          half * 128:(half + 1) * 128]
                            if corner == 0 and g < 11:
                                nc.scalar.activation(dst, srcg, Act.Copy,
                                                     scale=sc)
                            elif (corner == 0 and g >= 11) or \
                                 (corner == 1 and g < 4):
                                nc.gpsimd.tensor_scalar(dst, srcg, sc, None,
                                                        Alu.mult)
                            else:
                                nc.vector.tensor_scalar(dst, srcg, sc, None,
                                                        Alu.mult)
                        gl = (g % 4) * 128
                        for corner in range(4):
                            nc.tensor.matmul(
                                sTq[:, gl:gl + 128],
                                tbs[corner][:, g * 128:(g + 1) * 128],
                                id_sb[:, :],
                                start=(corner == 0), stop=(corner == 3))
                        sTqs[g // 4] = sTq
                        if g in (5, 9, 13, 15):
                            q = {5: 0, 9: 1, 13: 2, 15: 3}[g]
                            sl = slice(q * 512, (q + 1) * 512)
                            if q == 0:
                                nc.vector.tensor_copy(st16[:, sl],
                                                      sTqs[q][:, :])
                            else:
                                nc.scalar.activation(st16[:, sl], sTqs[q][:, :],
                                                     Act.Copy)
                            nc.tensor.matmul(
                                po[:, sl], wk_sb[:, k * O:(k + 1) * O],
                                st16[:, sl],
                                start=(k == 0), stop=(k == KK - 1))
                            if k == KK - 1:
                                if q == 0:
                                    nc.vector.tensor_copy(osb[:, sl],
                                                          po[:, sl])
                                else:
                                    nc.scalar.activation(osb[:, sl],
                                                         po[:, sl], Act.Copy)
                                nc.sync.dma_start(out[:, q * 512:(q + 1) * 512],
                                                  osb[:, sl])

    nc.compile()
    return nc


_CACHE = {}


def _get_nc(debug=False):
    key = bool(debug)
    if key not in _CACHE:
        nc = Bacc()
        _CACHE[key] = _build(nc, debug=debug)
    return _CACHE[key]


def _grid_tables(h0, order):
    """[128, 144] tables: [P, g*9+k] = gy/gx of (pixel, k) for the given
    slot->pixel order: 'A': pixel = g*128+P ; 'B': pixel = P*16+g."""
    ki = (np.arange(KK) // 3).astype(np.float32)
    kj = (np.arange(KK) % 3).astype(np.float32)
    P = np.arange(128)
    g = np.arange(G16)
    if order == "A":
        pix = g[None, :] * 128 + P[:, None]          # [128, 16]
    else:
        pix = P[:, None] * 16 + g[None, :]
    gy = (h0 + pix // W)[:, :, None] + (ki - 1.0)[None, None, :]
    gx = (pix % W)[:, :, None] + (kj - 1.0)[None, None, :]
    return (np.ascontiguousarray(gy.reshape(128, 144).astype(np.float32)),
            np.ascontiguousarray(gx.reshape(128, 144).astype(np.float32)))


def _prep_inputs(x, w_off, b_off, w_mask, b_mask, weight, bias):
    x = np.asarray(x, np.float32)
    w_off = np.asarray(w_off, np.float32)
    b_off = np.asarray(b_off, np.float32)
    w_mask = np.asarray(w_mask, np.float32)
    b_mask = np.asarray(b_mask, np.float32)
    weight = np.asarray(weight, np.float32)

    w_cat = np.concatenate([w_off[0::2], w_off[1::2], w_mask], axis=0)
    b_cat = np.concatenate([b_off[0::2], b_off[1::2], b_mask])
    wc = np.ascontiguousarray(
        w_cat.reshape(27, C, 9).transpose(1, 2, 0).reshape(C, 9 * 27)).astype(np.float16)
    bvec = np.ascontiguousarray(b_cat.reshape(27, 1))
    wkt = np.ascontiguousarray(
        weight.reshape(O, C, KK).transpose(1, 2, 0).reshape(C, KK * O)).astype(np.float16)
    ident = np.eye(128, dtype=np.float16)
    identf = np.eye(128, dtype=np.float32)

    in_maps = []
    for core in range(8):
        b = core // 2
        ph = core % 2
        h0 = ph * HROWS
        hl = h0 - 3
        xb = x[b].reshape(C, H, W)
        xhh = np.zeros((C, XR, W), np.float32)
        for r in range(XR):
            gr = hl + r
            if 0 <= gr < H:
                xhh[:, r] = xb[:, gr]
        # host-padded conv input: local halo rows 2..36 -> [C, 34*66]
        xpd = np.zeros((C, 34, 66), np.float32)
        xpd[:, :, 1:65] = xhh[:, 2:36, :]
        # gather source: x^T halo rows, [UR, 128] f16, zero rows 0 / UR-1
        xgat = np.zeros((UR, 128), np.float16)
        xgat[1:XPIX + 1, :C] = xhh.reshape(C, XPIX).T.astype(np.float16)
        gyA, gxA = _grid_tables(h0, "A")
        gyB, gxB = _grid_tables(h0, "B")
        gyB = gyB - 0.5
        gxB = gxB - 0.5
        shiftv = np.stack([np.full(128, 1.0 - hl * 64.0, np.float32),
                           np.full(128, 65.0 - hl * 64.0, np.float32)], axis=1)
        in_maps.append({
            "xp": np.ascontiguousarray(xpd.reshape(C, XPD)).astype(np.float16),
            "xg": np.ascontiguousarray(xgat.reshape(UR * 128)),
            "wc": wc, "bvec": bvec, "wkt": wkt,
            "gyA": gyA, "gxA": gxA, "gyB": gyB, "gxB": gxB,
            "shiftv": shiftv, "ident": ident, "identf": identf,
        })
    return in_maps


def kernel(x, w_off, b_off, w_mask, b_mask, weight, bias, _debug=False, _trace=False):
    nc = _get_nc(debug=_debug)
    in_maps = _prep_inputs(x, w_off, b_off, w_mask, b_mask, weight, bias)
    res = bass_utils.run_bass_kernel_spmd(
        nc, in_maps, core_ids=list(range(8)), trace=_trace)
    out = np.zeros((B, O, H, W), np.float32)
    for core in range(8):
        b, ph = core // 2, core % 2
        chunk = res.results[core]["out"]        # [O, HALF]
        out[b, :, ph * HROWS:(ph + 1) * HROWS, :] = chunk.reshape(O, HROWS, W)
    out += np.asarray(bias, np.float32)[None, :, None, None]
    if _debug or _trace:
        kernel._last = res
    return out


# revision 54
# speedup vs baseline: 1.0102x; 1.0102x over previous
"""Deformable Conv2d (DCNv2-style) Trainium2 Bass kernel.

Shards over 8 NeuronCores: core = b * 2 + ph  (b = batch 0..3, ph = pixel half).
Each core computes output pixels [ph*2048, (ph+1)*2048) of batch b.

Device pipeline per core (x-gather scheme; samples raw x rows, so there is
no U staging and the 52us of gather DMA is the only large HBM stream):
  1. offset/mask 3x3 conv as 9 shifted-tap matmuls (PE, f32, PSUM
     accumulate) over a host-padded input; junk PE transposes beforehand
     keep the cost model's PE p-state ramp at full clock for the conv
  2. B-order ([P, pixel P*16+g]) conv rows assembled via [18, 64]
     transposes per chunk-pair; B pipeline computes gather row indices
     (floor via RNE int cast, -0.5 baked into host tables); idx wrap
     ([16, n/16]) via PE transposes, then a 3-part DRAM bounce replicates
     it to 128 partitions so the first gathers start early.  A pipeline
     (bilinear coefficients cT, pixel-major) overlaps the first gathers.
  3. per tap: dma_gather of 4096 row-pair descriptors (512B) from the
     host-staged x^T halo (xg [UR*128] f16); the next tap's Pool-side
     descriptor prep is issued ahead of this tap's Pool scale ops so the
     DMA engines stay back-to-back; the last tap is split by y-pair to
     shorten the drain
  4. per group: scale the 4 bilinear corners (12 ACT / 8 Pool / 44 DVE
     per tap, per-partition scalars from cT); transpose-accumulate the
     scaled [pix, c] tiles into rotating PSUM quarters sT[c, pix] via
     matmul-with-identity (fuses corner-sum + transpose)
  5. per quarter: sT -> SBUF f16 (DVE/ACT), out[o, pix] += W_k^T @ sT
     accumulated in PSUM over the 9 taps
  6. store channel-major [128, 2048] f32 per quarter; host adds bias and
     reassembles [O, 32, 64] per core.
"""
import sys

sys.path.insert(0, "/opt/trn_rl_repo")

import numpy as np

import concourse.mybir as mybir
from concourse.ap import AP
from concourse.bacc import Bacc
from concourse.tile import TileContext
from concourse import bass_utils

F32 = mybir.dt.float32
F16 = mybir.dt.float16
I32 = mybir.dt.int32
I16 = mybir.dt.int16
Alu = mybir.AluOpType
Act = mybir.ActivationFunctionType

B, C, H, W = 4, 128, 64, 64
O, K, KK = 128, 3, 9
HWp = H * W
HALF = HWp // 2              # 2048 pixels per core
HROWS = 32
XR = 38                      # local halo rows: global [h0-3, h0+35); |dy|<2 safe
XPIX = XR * W                # 2560
UR = XPIX + 2                # xg rows (zero rows 0 and UR-1)
G16 = HALF // 128            # 16 pixel groups
XPD = 34 * 66                # host-padded conv input cols


def _colsA(tile, row0, nrow=9):
    """A-pipeline view: [128, 16g x nrow] cols of cP (stride 27)."""
    t = tile[:, :]
    return AP(tensor=t.tensor, offset=t.offset + row0,
              ap=[t.ap[0], [27, G16], [1, nrow]])


def _colsB(tile, row0, nrow=9):
    t = tile[:, :]
    return AP(tensor=t.tensor, offset=t.offset + row0,
              ap=[t.ap[0], [18, G16], [1, nrow]])


def _build(nc, debug=False):
    xp = nc.dram_tensor("xp", [C, XPD], F16, kind="ExternalInput")
    xg = nc.dram_tensor("xg", [UR * 128], F16, kind="ExternalInput")
    wc = nc.dram_tensor("wc", [C, 9 * 27], F16, kind="ExternalInput")
    bvec = nc.dram_tensor("bvec", [27, 1], F32, kind="ExternalInput")
    wkt = nc.dram_tensor("wkt", [C, KK * O], F16, kind="ExternalInput")
    gyA = nc.dram_tensor("gyA", [128, 144], F32, kind="ExternalInput")
    gxA = nc.dram_tensor("gxA", [128, 144], F32, kind="ExternalInput")
    gyB = nc.dram_tensor("gyB", [128, 144], F32, kind="ExternalInput")
    gxB = nc.dram_tensor("gxB", [128, 144], F32, kind="ExternalInput")
    shiftv = nc.dram_tensor("shiftv", [128, 2], F32, kind="ExternalInput")
    ident = nc.dram_tensor("ident", [128, 128], F16, kind="ExternalInput")
    identf = nc.dram_tensor("identf", [128, 128], F32, kind="ExternalInput")
    out = nc.dram_tensor("out", [O, HALF], F32, kind="ExternalOutput")

    idxstage = nc.dram_tensor("idxstage", [16 * 18 * 128], I16, kind="Internal")

    if debug:
        dbg_conv = nc.dram_tensor("dbg_conv", [27, HALF], F32, kind="ExternalOutput")
        dbg_cT = nc.dram_tensor("dbg_cT", [128, G16 * 36], F32, kind="ExternalOutput")
        dbg_wrap = nc.dram_tensor("dbg_wrap", [16, 18 * 128], I16, kind="ExternalOutput")

    with TileContext(nc) as tc:
        with (
            tc.tile_pool(name="big", bufs=1) as big,
            tc.tile_pool(name="small", bufs=1) as small,
        ):
            # idents first on the sync queue: the PE p-state filler is gated
            # only on idf_sb, so it must arrive as early as possible
            idf_sb = small.tile([128, 128], F32, tag="identf")
            nc.sync.dma_start(idf_sb[:, :], identf[:, :])
            id_sb = small.tile([128, 128], F16, tag="ident")
            nc.sync.dma_start(id_sb[:, :], ident[:, :])
            xpad = big.tile([C, XPD], F16, tag="xpad")
            nc.gpsimd.dma_start(xpad[:, :], xp[:, :])
            wc_sb = small.tile([C, 9 * 27], F16, tag="wc")
            nc.scalar.dma_start(wc_sb[:, :], wc[:, :])
            bvec_sb = small.tile([27, 1], F32, tag="bvec")
            nc.scalar.dma_start(bvec_sb[:, :], bvec[:, :])
            gyB_sb = small.tile([128, 144], F32, tag="gyB")
            nc.scalar.dma_start(gyB_sb[:, :], gyB[:, :])
            gxB_sb = small.tile([128, 144], F32, tag="gxB")
            nc.scalar.dma_start(gxB_sb[:, :], gxB[:, :])
            shift_sb = small.tile([128, 2], F32, tag="shiftv")
            nc.scalar.dma_start(shift_sb[:, :], shiftv[:, :])
            wk_sb = big.tile([C, KK * O], F16, tag="wk")
            nc.sync.dma_start(wk_sb[:, :], wkt[:, :])
            gyA_sb = small.tile([128, 144], F32, tag="gyA")
            nc.sync.dma_start(gyA_sb[:, :], gyA[:, :])
            gxA_sb = small.tile([128, 144], F32, tag="gxA")
            nc.sync.dma_start(gxA_sb[:, :], gxA[:, :])
            # dummy sigmoid (memset input, no DMA dep) so the first ACT
            # table load picks a set containing sigmoid+identity+copy
            sgdum = small.tile([27, 2], F32, tag="sgdum")
            nc.vector.memset(sgdum[:, 0:1], 0.0)
            nc.scalar.activation(sgdum[:, 1:2], sgdum[:, 0:1], Act.Sigmoid)

            # ---------- offset/mask conv (A order: col = pixel) ----------
            convR = big.tile([27, HALF], F32, tag="convR")
            with (
                tc.tile_pool(name="pfconv", bufs=2, space="PSUM") as psf,
                tc.tile_pool(name="pfb", bufs=1, space="PSUM") as psb,
                tc.tile_pool(name="pftr", bufs=4, space="PSUM") as psc,
            ):
                # PE p-state filler: junk transposes gated only on the early
                # ident load, keeping the PE busy-run start well before the
                # conv release so the conv matmuls dispatch at full clock.
                for _ in range(17):
                    pjunk = psc.tile([128, 128], F32, tag="tx")
                    nc.tensor.transpose(pjunk[0:16, :], idf_sb[:, 0:16],
                                        idf_sb[:, :])
                cRv = convR[:, :]
                # B-order rows: ptB_h[h][p64, g*18+row] = conv value (row)
                # at pixel (64h+p64)*16+g; filled per conv chunk-pair by
                # [18, 64] transposes (transpose outs must sit at PSUM
                # partition 0), then assembled into SBUF cPB by two copies.
                ptB0 = psb.tile([64, G16 * 18], F32, tag="ptB0")
                ptB1 = psb.tile([64, G16 * 18], F32, tag="ptB1")
                cPB = big.tile([128, G16 * 18], F32, tag="cPB")
                cP = big.tile([128, G16 * 27], F32, tag="cP")

                def emit_btrans(h2):
                    pt_h = ptB0 if h2 == 0 else ptB1
                    for g in range(G16):
                        nc.tensor.transpose(
                            pt_h[:, g * 18:(g + 1) * 18],
                            AP(tensor=cRv.tensor,
                               offset=cRv.offset + 1024 * h2 + g,
                               ap=[[cRv.ap[0][0], 18], [16, 64]]),
                            idf_sb[0:18, 0:18])
                    nc.vector.tensor_copy(
                        cPB[64 * h2:64 * h2 + 64, :], pt_h[:, :])

                def emit_atrans(ch):
                    for g in range(4 * ch, 4 * ch + 4):
                        pt = psc.tile([128, 128], F32, tag="tx")
                        nc.tensor.transpose(
                            pt[:, 0:27], convR[:, g * 128:(g + 1) * 128],
                            idf_sb[0:27, 0:27])
                        nc.scalar.activation(cP[:, g * 27:(g + 1) * 27],
                                             pt[:, 0:27], Act.Copy)

                for ch in range(4):
                    pc = psf.tile([27, 512], F32, tag="pf")
                    for th in range(3):
                        for tw in range(3):
                            tap = th * 3 + tw
                            rhs = AP(
                                tensor=xpad.tensor,
                                offset=xpad[:, :].offset + (ch * 8 + th) * 66 + tw,
                                ap=[xpad[:, :].ap[0], [66, 8], [1, W]],
                            )
                            nc.tensor.matmul(
                                pc[:, :], wc_sb[:, tap * 27:(tap + 1) * 27], rhs,
                                start=(tap == 0), stop=(tap == 8),
                            )
                    nc.scalar.activation(
                        convR[0:27, ch * 512:(ch + 1) * 512], pc[:, :], Act.Identity,
                        bias=bvec_sb[:, 0:1], scale=1.0,
                    )
                    if ch == 2:
                        emit_btrans(0)
                emit_btrans(1)

                # ---------- B pipeline: gather indices ----------
                bw = big.tile([128, 8 * 144], F32, tag="bw")
                idxPM = big.tile([128, 288], F32, tag="idxPM")
                itmpB = small.tile([128, 144], I32, tag="itmpB")
                itmpB2 = small.tile([128, 144], I32, tag="itmpB2")

                def Sb(q):
                    return bw[:, q * 144:(q + 1) * 144]

                BPY, BPX, BY0, BX0, BT, BIX, BCY, BT2 = range(8)
                # gyB/gxB tables have -0.5 pre-baked (RNE int cast = floor)
                nc.vector.tensor_tensor(Sb(BPY), _colsB(cPB, 0), gyB_sb[:, :], Alu.add)
                nc.vector.tensor_tensor(Sb(BPX), _colsB(cPB, 9), gxB_sb[:, :], Alu.add)
                nc.vector.tensor_copy(itmpB[:, :], Sb(BPY))
                nc.vector.tensor_copy(Sb(BY0), itmpB[:, :])
                nc.vector.tensor_copy(itmpB2[:, :], Sb(BPX))
                nc.vector.tensor_copy(Sb(BX0), itmpB2[:, :])
                nc.vector.tensor_scalar(Sb(BIX), Sb(BX0), -1.0, 63.0, Alu.max, Alu.min)

                def idx_view(pair):
                    t = idxPM[:, :]
                    return AP(tensor=t.tensor, offset=t.offset + pair * 144,
                              ap=[t.ap[0], [1, G16], [16, 9]])

                # per k-third (k0-2 | k3-5 | k6-8) so wraps/bounce start early
                def ksl(base, kg):
                    t = bw[:, :]
                    return AP(tensor=t.tensor,
                              offset=t.offset + base * 144 + kg * 3,
                              ap=[t.ap[0], [9, G16], [1, 3]])

                def idx_view_k(pair, kg):
                    t = idxPM[:, :]
                    return AP(tensor=t.tensor,
                              offset=t.offset + pair * 144 + kg * 48,
                              ap=[t.ap[0], [1, G16], [16, 3]])
                # idxPM col = pair*144 + k*16 + g ; k = 3kg+j -> offset 48kg ✓

                for kg in range(3):
                    # idx0 = clamp(y0)*64 + shift + ix
                    nc.vector.tensor_scalar(ksl(BCY, kg), ksl(BY0, kg),
                                            0.0, 63.0, Alu.max, Alu.min)
                    nc.vector.tensor_scalar(ksl(BT2, kg), ksl(BCY, kg), 64.0,
                                            shift_sb[:, 0:1], Alu.mult, Alu.add)
                    nc.vector.tensor_tensor(ksl(BT2, kg), ksl(BT2, kg),
                                            ksl(BIX, kg), Alu.add)
                    nc.vector.tensor_scalar(idx_view_k(0, kg), ksl(BT2, kg),
                                            0.0, float(UR - 2), Alu.max, Alu.min)
                    # idx1 = clamp(y0,-1,62)*64 + (shift+64) + ix
                    nc.vector.tensor_scalar(ksl(BCY, kg), ksl(BY0, kg),
                                            -1.0, 62.0, Alu.max, Alu.min)
                    nc.vector.tensor_scalar(ksl(BT2, kg), ksl(BCY, kg), 64.0,
                                            shift_sb[:, 1:2], Alu.mult, Alu.add)
                    nc.vector.tensor_tensor(ksl(BT2, kg), ksl(BT2, kg),
                                            ksl(BIX, kg), Alu.add)
                    nc.vector.tensor_scalar(idx_view_k(1, kg), ksl(BT2, kg),
                                            0.0, float(UR - 2), Alu.max, Alu.min)

                # idx transposes -> wrap rows [16, 128] each, cast to i16
                wrapS = big.tile([16, 18 * 128], I16, tag="wrapS")
                for k in range(KK):
                    for pair in range(2):
                        pw = psc.tile([128, 128], F32, tag="tx")
                        nc.tensor.transpose(
                            pw[0:16, :],
                            idxPM[:, pair * 144 + k * 16: pair * 144 + (k + 1) * 16],
                            idf_sb[:, :])
                        r = k * 2 + pair
                        if pair == 0:
                            nc.vector.tensor_copy(
                                wrapS[:, r * 128:(r + 1) * 128], pw[0:16, :])
                        else:
                            nc.scalar.activation(
                                wrapS[:, r * 128:(r + 1) * 128], pw[0:16, :], Act.Copy)
                if debug:
                    nc.sync.dma_start(dbg_wrap[:, :], wrapS[:, :])
                # bounce to DRAM and back replicated x8, in 3 k-thirds
                # so the first gathers start as early as possible
                idxW = big.tile([128, 18 * 128], I16, tag="idxW")
                for kg in range(3):
                    c0, c1 = kg * 768, (kg + 1) * 768
                    nc.sync.dma_start(
                        AP(tensor=idxstage, offset=c0,
                           ap=[[2304, 16], [1, 768]]),
                        wrapS[:, c0:c1])
                    nc.scalar.dma_start(
                        idxW[:, c0:c1],
                        AP(tensor=idxstage, offset=c0,
                           ap=[[0, 8], [2304, 16], [1, 768]]))

                # ---------- A pipeline: coefficients (pixel-major) ----------
                for ch in range(4):
                    emit_atrans(ch)

                NSL = 20
                cw = big.tile([128, NSL * 144], F32, tag="cw")
                itmp = small.tile([128, 144], I32, tag="itmp")
                cT = big.tile([128, G16 * 36], F32, tag="cT")

                def S(q):
                    return cw[:, q * 144:(q + 1) * 144]

                PY, PX, M, Y0, X0, FY, FX, Y1, X1 = range(9)
                CY0, CY1, VY0, VY1, VX0, VX1, IXC, T1, T2, T3 = range(9, 19)
                nc.vector.tensor_tensor(S(PY), _colsA(cP, 0), gyA_sb[:, :], Alu.add)
                nc.vector.tensor_tensor(S(PX), _colsA(cP, 9), gxA_sb[:, :], Alu.add)
                nc.scalar.activation(S(M), _colsA(cP, 18), Act.Sigmoid)
                # floors
                nc.vector.tensor_scalar(S(T1), S(PY), -0.5, None, Alu.add)
                nc.vector.tensor_copy(itmp[:, :], S(T1))
                nc.vector.tensor_copy(S(Y0), itmp[:, :])
                nc.vector.tensor_scalar(S(T1), S(PX), -0.5, None, Alu.add)
                nc.vector.tensor_copy(itmp[:, :], S(T1))
                nc.vector.tensor_copy(S(X0), itmp[:, :])
                nc.vector.tensor_tensor(S(FY), S(PY), S(Y0), Alu.subtract)
                nc.vector.tensor_tensor(S(FX), S(PX), S(X0), Alu.subtract)
                nc.vector.tensor_scalar(S(Y1), S(Y0), 1.0, None, Alu.add)
                nc.vector.tensor_scalar(S(X1), S(X0), 1.0, None, Alu.add)
                # validity
                nc.vector.tensor_scalar(S(CY0), S(Y0), 0.0, 63.0, Alu.max, Alu.min)
                nc.vector.tensor_tensor(S(VY0), S(CY0), S(Y0), Alu.is_equal)
                nc.vector.tensor_scalar(S(CY1), S(Y1), 0.0, 63.0, Alu.max, Alu.min)
                nc.vector.tensor_tensor(S(VY1), S(CY1), S(Y1), Alu.is_equal)
                nc.vector.tensor_scalar(S(T1), S(X0), 0.0, 63.0, Alu.max, Alu.min)
                nc.vector.tensor_tensor(S(VX0), S(T1), S(X0), Alu.is_equal)
                nc.vector.tensor_scalar(S(T1), S(X1), 0.0, 63.0, Alu.max, Alu.min)
                nc.vector.tensor_tensor(S(VX1), S(T1), S(X1), Alu.is_equal)
                # weights: wy0=(1-fy)*m*vy0 ; wy1=fy*m*vy1 ; ax0=(1-fx)*vx0 ; ax1=fx*vx1
                nc.vector.tensor_scalar(S(T1), S(FY), -1.0, 1.0, Alu.mult, Alu.add)
                nc.vector.tensor_tensor(S(T1), S(T1), S(M), Alu.mult)
                nc.vector.tensor_tensor(S(T1), S(T1), S(VY0), Alu.mult)     # wy0
                nc.vector.tensor_tensor(S(T2), S(FY), S(M), Alu.mult)
                nc.vector.tensor_tensor(S(T2), S(T2), S(VY1), Alu.mult)     # wy1
                nc.vector.tensor_scalar(S(T3), S(FX), -1.0, 1.0, Alu.mult, Alu.add)
                nc.vector.tensor_tensor(S(T3), S(T3), S(VX0), Alu.mult)     # ax0
                nc.vector.tensor_tensor(S(FX), S(FX), S(VX1), Alu.mult)     # ax1

                def cT_view(corner):
                    t = cT[:, :]
                    return AP(tensor=t.tensor, offset=t.offset + corner * 9,
                              ap=[t.ap[0], [36, G16], [1, 9]])

                nc.vector.tensor_tensor(cT_view(0), S(T1), S(T3), Alu.mult)  # c00
                nc.vector.tensor_tensor(cT_view(1), S(T1), S(FX), Alu.mult)  # c01
                nc.vector.tensor_tensor(cT_view(2), S(T2), S(T3), Alu.mult)  # c10
                nc.vector.tensor_tensor(cT_view(3), S(T2), S(FX), Alu.mult)  # c11
                if debug:
                    nc.sync.dma_start(dbg_cT[:, :], cT[:, :])

            # ---------- gathers + combine ----------
            src_ap = AP(tensor=xg, offset=0, ap=[[128, UR - 1], [1, 256]])
            with (
                tc.tile_pool(name="gat", bufs=4) as gat,
                tc.tile_pool(name="tmul", bufs=10) as tmul,
                tc.tile_pool(name="pst", bufs=4, space="PSUM") as pst,
                tc.tile_pool(name="pso", bufs=1, space="PSUM") as pso,
                tc.tile_pool(name="stsb", bufs=3) as stsb,
                tc.tile_pool(name="osb", bufs=1) as osbp,
            ):
                po = pso.tile([128, HALF], F32, tag="po")
                osb = osbp.tile([128, HALF], F32, tag="osb")
                gts = []

                def emit_gather(k):
                    gt = gat.tile([128, 2 * G16, 256], F16, tag="gt")
                    nc.gpsimd.dma_gather(
                        gt[:, :, :], src_ap,
                        idxW[:, k * 256:(k + 1) * 256],
                        2 * HALF, 2 * HALF, 256, elem_step=128,
                        single_packet=False)
                    gts.append(gt)

                def emit_gather8(pair):
                    # last tap split by y-corner pair: its combine can start
                    # after the first half lands, shortening the tail
                    gt = gat.tile([128, G16, 256], F16, tag="gt8")
                    c0 = (KK - 1) * 256 + pair * 128
                    nc.gpsimd.dma_gather(
                        gt[:, :, :], src_ap,
                        idxW[:, c0:c0 + 128],
                        HALF, HALF, 256, elem_step=128,
                        single_packet=False)
                    gts.append(gt)

                emit_gather(0)
                for k in range(KK):
                    # issue the next gather's Pool prep before this tap's
                    # Pool scale ops so the DMA cadence never head-of-line
                    # blocks on data-dependent work
                    if k + 1 < KK - 1:
                        emit_gather(k + 1)
                    elif k + 1 == KK - 1:
                        emit_gather8(0)
                        emit_gather8(1)
                    gt = gts[k]
                    st16 = stsb.tile([128, HALF], F16, tag="st16")
                    tbs = []
                    for _c in range(4):
                        tb = tmul.tile([128, HALF], F16, tag="tb")
                        tbs.append(tb)
                    # per group: scale the 4 corners (ACT first, then DVE),
                    # transpose-accumulate into the quarter's sT[c, pix] via
                    # matmul-with-identity (consecutive accumulation group);
                    # per quarter: copy sT -> f16 and run the W matmul.
                    # sT quarters rotate through 4 PSUM banks so tap k+1's
                    # transposes only wait on this tap's same-quarter copy.
                    sTq = None
                    sTqs = [None] * 4
                    for g in range(G16):
                        if g % 4 == 0:
                            sTq = pst.tile([128, 512], F32, tag="sTq")
                        for corner in (0, 1, 2, 3):
                            sc = cT[:, g * 36 + corner * 9 + k:
                                    g * 36 + corner * 9 + k + 1]
                            dst = tbs[corner][:, g * 128:(g + 1) * 128]
                            pair, half = corner // 2, corner % 2
                            if k == KK - 1:
                                srcg = gts[KK - 1 + pair][:, g,
                                           half * 128:(half + 1) * 128]
                            else:
                                srcg = gt[:, pair * G16 + g,
                                          half * 128:(half + 1) * 128]
                            if corner == 0 and g < 12:
                                nc.scalar.activation(dst, srcg, Act.Copy,
                                                     scale=sc)
                            elif (corner == 0 and g >= 12) or \
                                 (corner == 1 and g < 4):
                                nc.gpsimd.tensor_scalar(dst, srcg, sc, None,
                                                        Alu.mult)
                            else:
                                nc.vector.tensor_scalar(dst, srcg, sc, None,
                                                        Alu.mult)
                        gl = (g % 4) * 128
                        for corner in range(4):
                            nc.tensor.matmul(
                                sTq[:, gl:gl + 128],
                                tbs[corner][:, g * 128:(g + 1) * 128],
                                id_sb[:, :],
                                start=(corner == 0), stop=(corner == 3))
                        sTqs[g // 4] = sTq
                        if g in (5, 9, 13, 15):
                            q = {5: 0, 9: 1, 13: 2, 15: 3}[g]
                            sl = slice(q * 512, (q + 1) * 512)
                            if q == 0:
                                nc.vector.tensor_copy(st16[:, sl],
                                                      sTqs[q][:, :])
                            else:
                                nc.scalar.activation(st16[:, sl], sTqs[q][:, :],
                                                     Act.Copy)
                            nc.tensor.matmul(
                                po[:, sl], wk_sb[:, k * O:(k + 1) * O],
                                st16[:, sl],
                                start=(k == 0), stop=(k == KK - 1))
                            if k == KK - 1:
                                if q == 0:
                                    nc.vector.tensor_copy(osb[:, sl],
                                                          po[:, sl])
                                else:
                                    nc.scalar.activation(osb[:, sl],
                                                         po[:, sl], Act.Copy)
                                nc.sync.dma_start(out[:, q * 512:(q + 1) * 512],
                                                  osb[:, sl])

    nc.compile()
    return nc


_CACHE = {}


def _get_nc(debug=False):
    key = bool(debug)
    if key not in _CACHE:
        nc = Bacc()
        _CACHE[key] = _build(nc, debug=debug)
    return _CACHE[key]


def _grid_tables(h0, order):
    """[128, 144] tables: [P, g*9+k] = gy/gx of (pixel, k) for the given
    slot->pixel order: 'A': pixel = g*128+P ; 'B': pixel = P*16+g."""
    ki = (np.arange(KK) // 3).astype(np.float32)
    kj = (np.arange(KK) % 3).astype(np.float32)
    P = np.arange(128)
    g = np.arange(G16)
    if order == "A":
        pix = g[None, :] * 128 + P[:, None]          # [128, 16]
    else:
        pix = P[:, None] * 16 + g[None, :]
    gy = (h0 + pix // W)[:, :, None] + (ki - 1.0)[None, None, :]
    gx = (pix % W)[:, :, None] + (kj - 1.0)[None, None, :]
    return (np.ascontiguousarray(gy.reshape(128, 144).astype(np.float32)),
            np.ascontiguousarray(gx.reshape(128, 144).astype(np.float32)))


def _prep_inputs(x, w_off, b_off, w_mask, b_mask, weight, bias):
    x = np.asarray(x, np.float32)
    w_off = np.asarray(w_off, np.float32)
    b_off = np.asarray(b_off, np.float32)
    w_mask = np.asarray(w_mask, np.float32)
    b_mask = np.asarray(b_mask, np.float32)
    weight = np.asarray(weight, np.float32)

    w_cat = np.concatenate([w_off[0::2], w_off[1::2], w_mask], axis=0)
    b_cat = np.concatenate([b_off[0::2], b_off[1::2], b_mask])
    wc = np.ascontiguousarray(
        w_cat.reshape(27, C, 9).transpose(1, 2, 0).reshape(C, 9 * 27)).astype(np.float16)
    bvec = np.ascontiguousarray(b_cat.reshape(27, 1))
    wkt = np.ascontiguousarray(
        weight.reshape(O, C, KK).transpose(1, 2, 0).reshape(C, KK * O)).astype(np.float16)
    ident = np.eye(128, dtype=np.float16)
    identf = np.eye(128, dtype=np.float32)

    in_maps = []
    for core in range(8):
        b = core // 2
        ph = core % 2
        h0 = ph * HROWS
        hl = h0 - 3
        xb = x[b].reshape(C, H, W)
        xhh = np.zeros((C, XR, W), np.float32)
        for r in range(XR):
            gr = hl + r
            if 0 <= gr < H:
                xhh[:, r] = xb[:, gr]
        # host-padded conv input: local halo rows 2..36 -> [C, 34*66]
        xpd = np.zeros((C, 34, 66), np.float32)
        xpd[:, :, 1:65] = xhh[:, 2:36, :]
        # gather source: x^T halo rows, [UR, 128] f16, zero rows 0 / UR-1
        xgat = np.zeros((UR, 128), np.float16)
        xgat[1:XPIX + 1, :C] = xhh.reshape(C, XPIX).T.astype(np.float16)
        gyA, gxA = _grid_tables(h0, "A")
        gyB, gxB = _grid_tables(h0, "B")
        gyB = gyB - 0.5
        gxB = gxB - 0.5
        shiftv = np.stack([np.full(128, 1.0 - hl * 64.0, np.float32),
                           np.full(128, 65.0 - hl * 64.0, np.float32)], axis=1)
        in_maps.append({
            "xp": np.ascontiguousarray(xpd.reshape(C, XPD)).astype(np.float16),
            "xg": np.ascontiguousarray(xgat.reshape(UR * 128)),
            "wc": wc, "bvec": bvec, "wkt": wkt,
            "gyA": gyA, "gxA": gxA, "gyB": gyB, "gxB": gxB,
            "shiftv": shiftv, "ident": ident, "identf": identf,
        })
    return in_maps


def kernel(x, w_off, b_off, w_mask, b_mask, weight, bias, _debug=False, _trace=False):
    nc = _get_nc(debug=_debug)
    in_maps = _prep_inputs(x, w_off, b_off, w_mask, b_mask, weight, bias)
    res = bass_utils.run_bass_kernel_spmd(
        nc, in_maps, core_ids=list(range(8)), trace=_trace)
    out = np.zeros((B, O, H, W), np.float32)
    for core in range(8):
        b, ph = core // 2, core % 2
        chunk = res.results[core]["out"]        # [O, HALF]
        out[b, :, ph * HROWS:(ph + 1) * HROWS, :] = chunk.reshape(O, HROWS, W)
    out += np.asarray(bias, np.float32)[None, :, None, None]
    if _debug or _trace:
        kernel._last = res
    return out
